# revision 11
# baseline (speedup 1.0000x reference)
"""BiLSTM encoder kernel for 8 Trainium2 NeuronCores.

Problem: N=32, S=256, D=1024, H=1024 bidirectional LSTM.

Sharding (no cross-core communication):
  8 cores = 2 directions x 4 batch-quarters (8 batch rows each).
  Each core:
    Phase 1 (proj): x_h[token, col] = x^T-stationary @ W_ih-streaming  (+bias)
    Phase 2 (scan): 256 sequential LSTM steps.
      Matmul orientation: h^T stationary ([128,8] per K-chunk), W_hh streams
      through 4 concurrent PE column-groups (tile_position) -> stream-bound.
      x_h[t] enters the same PSUM accumulation via a K=8 identity matmul.
      Elementwise on junk-padded [128, F] tiles (cost is free-dim driven).
      h' is re-transposed for the next step with 2 PE transposes.

Device gate-column permutation: col = 1024*q + 512*half + 128*gate + j
  with H_global = 256*q + 128*half + j  (q: PE col-group, gate: f,i,o,g).

All 8 cores run an identical program; direction/batch specialization is
done host-side (time-reversed inputs for the backward cores).
"""

import os
import sys

for _p in ("/opt/trn_rl_repo", "/root/.axon_site/_ro/trn_rl_repo"):
    if os.path.isdir(_p) and _p not in sys.path:
        sys.path.insert(0, _p)

import numpy as np

import concourse.bass as bass
import concourse.mybir as mybir
from concourse import bacc
from concourse import bass_utils
from concourse.tile import TileContext
from concourse.masks import make_identity

N, S, D, H = 32, 256, 1024, 1024
B = 8            # batch rows per core
NCORES = 8
TOK = B * S      # tokens per core (t-major: token = t*B + b)
GC = 4 * H       # gate columns per direction (f,i,o,g x H)

F32 = mybir.dt.float32
F16 = mybir.dt.float16


def build_nc(steps=S):
    """Build and schedule the SPMD program (identical on all 8 cores)."""
    nc = bacc.Bacc(
        "TRN2",
        target_bir_lowering=False,
        debug=False,
        enable_asserts=False,
        num_devices=NCORES,
    )

    ntok = B * steps
    xT = nc.dram_tensor("xT", [D, ntok], F16, kind="ExternalInput").ap()
    wproj = nc.dram_tensor("wproj", [D, GC], F16, kind="ExternalInput").ap()
    bias_d = nc.dram_tensor("bias", [1, GC], F16, kind="ExternalInput").ap()
    whh = nc.dram_tensor("whh", [H, GC], F16, kind="ExternalInput").ap()
    h0T = nc.dram_tensor("h0T", [128, 256], F16, kind="ExternalInput").ap()
    c0 = nc.dram_tensor("c0", [128, 256], F32, kind="ExternalInput").ap()
    i832_d = nc.dram_tensor("i832", [8, 32], F16, kind="ExternalInput").ap()
    hs = nc.dram_tensor("hs", [steps, 4, B, 256], F16, kind="ExternalOutput").ap()

    byp = mybir.AluOpType.bypass
    mult = mybir.AluOpType.mult
    add = mybir.AluOpType.add
    SIG = mybir.ActivationFunctionType.Sigmoid
    TANH = mybir.ActivationFunctionType.Tanh

    # number of 128-token stationary blocks in proj
    n_tb = ntok // 128

    with TileContext(nc) as tc:
        with tc.tile_pool(name="dram", bufs=1, space="DRAM") as dpool:
            xh_dram = dpool.tile([ntok, GC], F16, tag="xh")

            # ---------------- Phase 1: input projection ----------------
            with tc.tile_pool(name="proj", bufs=1) as ppersist, \
                 tc.tile_pool(name="projw", bufs=1) as pw, \
                 tc.tile_pool(name="projps", bufs=2, space="PSUM") as pps, \
                 tc.tile_pool(name="projsb", bufs=2) as psb:
                xt_t = []
                for d in range(8):
                    t = ppersist.tile([128, ntok], F16, tag=f"xt{d}")
                    nc.sync.dma_start(t[:], xT[128 * d:128 * (d + 1), :])
                    xt_t.append(t)
                ones1 = ppersist.tile([1, 128], F16, tag="ones1")
                nc.vector.memset(ones1[:], 1.0)
                bias_sb = ppersist.tile([1, GC], F16, tag="biassb")
                nc.sync.dma_start(bias_sb[:], bias_d[:])

                for hf in range(2):  # column halves of GC (2048 each)
                    wt = []
                    for d in range(8):
                        t = pw.tile([128, 2048], F16, tag=f"wp{d}")
                        nc.sync.dma_start(
                            t[:], wproj[128 * d:128 * (d + 1),
                                        2048 * hf:2048 * (hf + 1)])
                        wt.append(t)
                    for tb in range(n_tb):
                        ps = pps.tile([128, 2048], F32, tag="projps")
                        for cb in range(4):
                            o = ps[:, 512 * cb:512 * (cb + 1)]
                            for d in range(8):
                                nc.tensor.matmul(
                                    o,
                                    (xt_t[d][:, 128 * tb:128 * (tb + 1)]),
                                    (wt[d][:, 512 * cb:512 * (cb + 1)]),
                                    start=(d == 0), stop=False)
                            nc.tensor.matmul(
                                o, (ones1[:, :]),
                                (bias_sb[:, 2048 * hf + 512 * cb:
                                            2048 * hf + 512 * (cb + 1)]),
                                start=False, stop=True)
                        sb = psb.tile([128, 2048], F16, tag="projsb")
                        nc.scalar.activation(
                            sb[:], ps[:], mybir.ActivationFunctionType.Copy)
                        nc.sync.dma_start(
                            xh_dram[128 * tb:128 * (tb + 1),
                                    2048 * hf:2048 * (hf + 1)], sb[:])

            # ---------------- Phase 2: recurrent scan ----------------
            with tc.tile_pool(name="scanw", bufs=1) as sw, \
                 tc.tile_pool(name="scanst", bufs=1) as sst, \
                 tc.tile_pool(name="xhp", bufs=2) as xhp, \
                 tc.tile_pool(name="gps", bufs=2, space="PSUM") as gps, \
                 tc.tile_pool(name="tps", bufs=2, space="PSUM") as tps, \
                 tc.tile_pool(name="selem", bufs=2) as selem, \
                 tc.tile_pool(name="hsp", bufs=3) as hsp, \
                 tc.tile_pool(name="lhsp", bufs=2) as lhsp:

                wt = []
                for c in range(8):
                    t = sw.tile([128, GC], F16, tag=f"whh{c}")
                    nc.sync.dma_start(t[:], whh[128 * c:128 * (c + 1), :])
                    wt.append(t)
                i832 = sst.tile([8, 32], F16, tag="i832")
                nc.sync.dma_start(i832[:], i832_d[:])
                ident = sst.tile([128, 128], F16, tag="ident")
                make_identity(nc, ident[:])
                cst = sst.tile([128, 256], F32, tag="cst")
                nc.sync.dma_start(cst[:], c0[:])

                LE = lhsp.tile([128, 128], F16, tag="LE")
                LO = lhsp.tile([128, 128], F16, tag="LO")
                nc.sync.dma_start(LE[:], h0T[:, 0:128])
                nc.sync.dma_start(LO[:], h0T[:, 128:256])

                for t in range(steps):
                    xh_t = xhp.tile([8, GC], F16, tag="xht")
                    nc.sync.dma_start(xh_t[:], xh_dram[B * t:B * (t + 1), :])

                    # MM order: col-group q innermost so the 4 PE column
                    # groups stream concurrently. Every MM writes all 32
                    # rows of its group block so accumulation groups are
                    # well-formed (junk rows carry bounded junk).
                    P = gps.tile([128, 1024], F32, tag="P")
                    for h2 in range(2):
                        for q in range(4):
                            cs = 1024 * q + 512 * h2
                            nc.tensor.matmul(
                                P[32 * q:32 * q + 32, 512 * h2:512 * (h2 + 1)],
                                (i832[:, :]),
                                (xh_t[:, cs:cs + 512]),
                                start=True, stop=False,
                                skip_group_check=True,
                                tile_position=(0, 32 * q))
                        for c in (0, 2, 4, 6, 1, 3, 5, 7):
                            lt = LE if (c % 2 == 0) else LO
                            for q in range(4):
                                cs = 1024 * q + 512 * h2
                                nc.tensor.matmul(
                                    P[32 * q:32 * q + 32,
                                      512 * h2:512 * (h2 + 1)],
                                    (lt[:, 32 * (c // 2):32 * (c // 2) + 32]),
                                    (wt[c][:, cs:cs + 512]),
                                    start=False, stop=(c == 7),
                                    skip_group_check=True,
                                    tile_position=(0, 32 * q))

                    Hs = hsp.tile([128, 256], F16, tag="Hs")
                    LE = lhsp.tile([128, 128], F16, tag="LE")
                    LO = lhsp.tile([128, 128], F16, tag="LO")
                    for hf in range(2):
                        pcol = 512 * hf
                        Sf = selem.tile([128, 384], F32, tag=f"S{hf}")
                        nc.scalar.activation(Sf[:], P[:, pcol:pcol + 384], SIG)
                        Sg = selem.tile([128, 128], F32, tag=f"Sg{hf}")
                        nc.scalar.activation(
                            Sg[:], P[:, pcol + 384:pcol + 512], TANH)
                        U = selem.tile([128, 128], F32, tag=f"U{hf}")
                        nc.vector.scalar_tensor_tensor(
                            U[:], Sf[:, 128:256], 1.0, Sg[:], byp, mult)
                        ch = cst[:, 128 * hf:128 * (hf + 1)]
                        nc.vector.scalar_tensor_tensor(
                            ch, Sf[:, 0:128], 1.0, ch, byp, mult)
                        nc.vector.scalar_tensor_tensor(
                            ch, ch, 1.0, U[:], byp, add)
                        Tc = selem.tile([128, 128], F32, tag=f"Tc{hf}")
                        nc.scalar.activation(Tc[:], ch, TANH)
                        hh = Hs[:, 128 * hf:128 * (hf + 1)]
                        nc.vector.scalar_tensor_tensor(
                            hh, Sf[:, 256:384], 1.0, Tc[:], byp, mult)
                        # transpose this half -> lhsT for next step
                        TT = tps.tile([128, 128], F16, tag=f"T{hf}")
                        nc.tensor.transpose(TT[:], hh, ident[:])
                        dst = LE if hf == 0 else LO
                        nc.vector.tensor_copy(dst[:], TT[:])

                    # write hidden state of this step to DRAM (one DMA per
                    # column-group: partition slices must be contiguous)
                    for q in range(4):
                        nc.sync.dma_start(hs[t, q], Hs[32 * q:32 * q + 8, :])

    nc.compile()
    return nc


_CACHE = {}


def _get_nc(steps=S):
    if steps not in _CACHE:
        _CACHE[steps] = build_nc(steps)
    return _CACHE[steps]


# Device gate-column permutation: col = 1024q + 512*half + 128*gate + j,
# H_global = 256q + 128*half + j
_COL = np.arange(GC)
_COL_Q = _COL // 1024
_COL_HALF = (_COL % 1024) // 512
_COL_GATE = (_COL % 512) // 128
_COL_J = _COL % 128
_COL_H = 256 * _COL_Q + 128 * _COL_HALF + _COL_J


def _prep_core(core, x, weight_ih, weight_hh, bias, h0_f, h0_b, c0_f, c0_b,
               steps=S):
    d = core // 4
    qt = core % 4
    fwd = (d == 0)

    xs = np.asarray(x[8 * qt:8 * qt + 8, :steps], np.float32)       # (8,steps,D)
    if not fwd:
        xs = xs[:, ::-1]
    xT = np.ascontiguousarray(xs.transpose(2, 1, 0).reshape(D, B * steps))

    wih = np.asarray(weight_ih[4 * d:4 * d + 4], np.float32)        # (4,D,H)
    whh = np.asarray(weight_hh[4 * d:4 * d + 4], np.float32)
    bi = np.asarray(bias[4 * d:4 * d + 4, 0], np.float32)           # (4,H)

    wproj = np.ascontiguousarray(wih[_COL_GATE, :, _COL_H].T)       # (D,GC)
    whh_dev = np.ascontiguousarray(whh[_COL_GATE, :, _COL_H].T)     # (H,GC)
    bias_dev = np.ascontiguousarray(bi[_COL_GATE, _COL_H])[None, :]

    h0 = np.asarray(h0_f if fwd else h0_b, np.float32).reshape(H)
    c0 = np.asarray(c0_f if fwd else c0_b, np.float32).reshape(H)

    h0T = np.zeros((128, 256), np.float32)
    p = np.arange(128)
    for q in range(4):
        for b in range(8):
            h0T[:, 32 * q + b] = h0[256 * q + p]
            h0T[:, 128 + 32 * q + b] = h0[256 * q + 128 + p]

    c0_dev = np.zeros((128, 256), np.float32)
    j = np.arange(128)
    for q in range(4):
        for b in range(8):
            c0_dev[32 * q + b, 0:128] = c0[256 * q + j]
            c0_dev[32 * q + b, 128:256] = c0[256 * q + 128 + j]

    i832 = np.zeros((8, 32), np.float32)
    i832[np.arange(8), np.arange(8)] = 1.0

    return {
        "xT": xT.astype(np.float16),
        "wproj": wproj.astype(np.float16),
        "bias": bias_dev.astype(np.float16),
        "whh": whh_dev.astype(np.float16),
        "h0T": h0T.astype(np.float16),
        "c0": c0_dev,
        "i832": i832.astype(np.float16),
    }


def prep_in_maps(x, mask, weight_ih, weight_hh, bias, h0_f, h0_b, c0_f, c0_b,
                 steps=S):
    return [
        _prep_core(c, x, weight_ih, weight_hh, bias, h0_f, h0_b, c0_f, c0_b,
                   steps)
        for c in range(NCORES)
    ]


def assemble(results, mask, steps=S):
    """results: list of 8 dicts with 'hs' (steps,4,B,256)."""
    hidden = np.zeros((N, steps, 2 * H), np.float32)
    for core in range(NCORES):
        d = core // 4
        qt = core % 4
        arr = np.asarray(results[core]["hs"]).astype(np.float32)  # (steps,4,8,256)
        part = arr.transpose(2, 0, 1, 3).reshape(B, steps, H)
        if d == 1:
            part = part[:, ::-1]
        hidden[8 * qt:8 * qt + 8, :, H * d:H * (d + 1)] = part
    final = np.concatenate([hidden[:, -1, :H], hidden[:, 0, H:]], axis=1)
    hidden = hidden * np.asarray(mask, np.float32)[:, :steps, None]
    return hidden.astype(np.float32), final.astype(np.float32)


def kernel(x, mask, weight_ih, weight_hh, bias, h0_f, h0_b, c0_f, c0_b):
    nc = _get_nc(S)
    in_maps = prep_in_maps(x, mask, weight_ih, weight_hh, bias,
                           h0_f, h0_b, c0_f, c0_b, S)
    res = bass_utils.run_bass_kernel_spmd(nc, in_maps,
                                          core_ids=list(range(NCORES)))
    return assemble(res.results, mask, S)


# revision 12
# speedup vs baseline: 1.1368x; 1.1368x over previous
"""BiLSTM encoder kernel for 8 Trainium2 NeuronCores.

Problem: N=32, S=256, D=1024, H=1024 bidirectional LSTM.

Sharding (no cross-core communication):
  8 cores = 2 directions x 4 batch-quarters (8 batch rows each).
  Each core:
    Phase 1 (proj): x_h[token, col] = x^T-stationary @ W_ih-streaming  (+bias)
    Phase 2 (scan): 256 sequential LSTM steps.
      Matmul orientation: h^T stationary ([128,8] per K-chunk), W_hh streams
      through 4 concurrent PE column-groups (tile_position) -> stream-bound.
      x_h[t] enters the same PSUM accumulation via a K=8 identity matmul.
      Elementwise on junk-padded [128, F] tiles (cost is free-dim driven).
      h' is re-transposed for the next step with 2 PE transposes.

Device gate-column permutation: col = 1024*q + 512*half + 128*gate + j
  with H_global = 256*q + 128*half + j  (q: PE col-group, gate: f,i,o,g).

All 8 cores run an identical program; direction/batch specialization is
done host-side (time-reversed inputs for the backward cores).
"""

import os
import sys

for _p in ("/opt/trn_rl_repo", "/root/.axon_site/_ro/trn_rl_repo"):
    if os.path.isdir(_p) and _p not in sys.path:
        sys.path.insert(0, _p)

import numpy as np

import concourse.bass as bass
import concourse.mybir as mybir
from concourse import bacc
from concourse import bass_utils
from concourse.tile import TileContext
from concourse.masks import make_identity

N, S, D, H = 32, 256, 1024, 1024
B = 8            # batch rows per core
NCORES = 8
TOK = B * S      # tokens per core (t-major: token = t*B + b)
GC = 4 * H       # gate columns per direction (f,i,o,g x H)

F32 = mybir.dt.float32
F16 = mybir.dt.float16


def build_nc(steps=S, use_bias=False):
    """Build and schedule the SPMD program (identical on all 8 cores).

    Single fused phase: the input projection is emitted as small
    "granules" (one token-block x 512-col block) interleaved into the
    scan loop, so they fill PE idle slots while the per-step elementwise
    chain runs, and keep the PE HAM-warm.
    """
    nc = bacc.Bacc(
        "TRN2",
        target_bir_lowering=False,
        debug=False,
        enable_asserts=False,
        num_devices=NCORES,
    )

    ntok = B * steps
    xT = nc.dram_tensor("xT", [D, ntok], F16, kind="ExternalInput").ap()
    wproj = nc.dram_tensor("wproj", [D, GC], F16, kind="ExternalInput").ap()
    if use_bias:
        bias_d = nc.dram_tensor("bias", [1, GC], F16, kind="ExternalInput").ap()
    whh = nc.dram_tensor("whh", [H, GC], F16, kind="ExternalInput").ap()
    h0T = nc.dram_tensor("h0T", [128, 256], F16, kind="ExternalInput").ap()
    c0 = nc.dram_tensor("c0", [128, 256], F32, kind="ExternalInput").ap()
    i832_d = nc.dram_tensor("i832", [8, 32], F16, kind="ExternalInput").ap()
    hs = nc.dram_tensor("hs", [steps, 4, B, 256], F16, kind="ExternalOutput").ap()

    byp = mybir.AluOpType.bypass
    mult = mybir.AluOpType.mult
    add = mybir.AluOpType.add
    SIG = mybir.ActivationFunctionType.Sigmoid
    TANH = mybir.ActivationFunctionType.Tanh

    n_tb = ntok // 128          # 128-token stationary blocks
    n_gran = n_tb * 8           # granules: (tb, hf, cb) with 512 cols each

    with TileContext(nc) as tc:
        with tc.tile_pool(name="dram", bufs=1, space="DRAM") as dpool, \
             tc.tile_pool(name="wts", bufs=1) as wts, \
             tc.tile_pool(name="sst", bufs=1) as sst, \
             tc.tile_pool(name="xhp", bufs=2) as xhp, \
             tc.tile_pool(name="gps", bufs=2, space="PSUM") as gps, \
             tc.tile_pool(name="tps", bufs=1, space="PSUM") as tps, \
             tc.tile_pool(name="pps", bufs=2, space="PSUM") as pps, \
             tc.tile_pool(name="psb", bufs=2) as psb, \
             tc.tile_pool(name="selem", bufs=2) as selem, \
             tc.tile_pool(name="hsp", bufs=3) as hsp, \
             tc.tile_pool(name="lhsp", bufs=2) as lhsp:

            xh_dram = dpool.tile([ntok, GC], F16, tag="xh")

            # ---- static tiles / weight loads ----
            xt_t = []
            for d in range(8):
                t = wts.tile([128, ntok], F16, tag=f"xt{d}")
                nc.sync.dma_start(t[:], xT[128 * d:128 * (d + 1), :])
                xt_t.append(t)
            wp_t = []
            for d in range(8):
                t = wts.tile([128, GC], F16, tag=f"wp{d}")
                nc.sync.dma_start(t[:], wproj[128 * d:128 * (d + 1), :])
                wp_t.append(t)
            wt = []
            for c in range(8):
                t = wts.tile([128, GC], F16, tag=f"whh{c}")
                nc.sync.dma_start(t[:], whh[128 * c:128 * (c + 1), :])
                wt.append(t)
            if use_bias:
                ones1 = sst.tile([1, 128], F16, tag="ones1")
                nc.vector.memset(ones1[:], 1.0)
                bias_sb = sst.tile([1, GC], F16, tag="biassb")
                nc.sync.dma_start(bias_sb[:], bias_d[:])
            i832 = sst.tile([8, 32], F16, tag="i832")
            nc.sync.dma_start(i832[:], i832_d[:])
            ident = sst.tile([128, 128], F16, tag="ident")
            make_identity(nc, ident[:])
            cst = sst.tile([128, 256], F32, tag="cst")
            nc.sync.dma_start(cst[:], c0[:])

            LE = lhsp.tile([128, 128], F16, tag="LE")
            LO = lhsp.tile([128, 128], F16, tag="LO")
            nc.sync.dma_start(LE[:], h0T[:, 0:128])
            nc.sync.dma_start(LO[:], h0T[:, 128:256])

            def proj_granule(g):
                tb = g // 8
                hf = (g % 8) // 4
                cb = g % 4
                col = 2048 * hf + 512 * cb
                pp = pps.tile([128, 512], F32, tag="pp")
                for d in range(8):
                    nc.tensor.matmul(
                        pp[:], xt_t[d][:, 128 * tb:128 * (tb + 1)],
                        wp_t[d][:, col:col + 512],
                        start=(d == 0), stop=(d == 7 and not use_bias))
                if use_bias:
                    nc.tensor.matmul(
                        pp[:], ones1[:, :], bias_sb[:, col:col + 512],
                        start=False, stop=True)
                sb = psb.tile([128, 512], F16, tag="projsb")
                nc.scalar.activation(
                    sb[:], pp[:], mybir.ActivationFunctionType.Copy)
                nc.sync.dma_start(
                    xh_dram[128 * tb:128 * (tb + 1), col:col + 512], sb[:])

            # prologue: first two token blocks of x_h
            n_pro = min(16, n_gran)
            for g in range(n_pro):
                proj_granule(g)

            for t in range(steps):
                xh_t = xhp.tile([8, GC], F16, tag="xht")
                nc.sync.dma_start(xh_t[:], xh_dram[B * t:B * (t + 1), :])

                # MM order: col-group q innermost so the 4 PE column groups
                # stream concurrently; even K-chunks (needing only LE)
                # before odd ones. Every MM writes all 32 rows of its
                # group block so accumulation groups are well-formed.
                P = gps.tile([128, 1024], F32, tag="P")
                for h2 in range(2):
                    for q in range(4):
                        cs = 1024 * q + 512 * h2
                        nc.tensor.matmul(
                            P[32 * q:32 * q + 32, 512 * h2:512 * (h2 + 1)],
                            i832[:, :], xh_t[:, cs:cs + 512],
                            start=True, stop=False,
                            skip_group_check=True,
                            tile_position=(0, 32 * q))
                    for c in (0, 2, 4, 6, 1, 3, 5, 7):
                        lt = LE if (c % 2 == 0) else LO
                        for q in range(4):
                            cs = 1024 * q + 512 * h2
                            nc.tensor.matmul(
                                P[32 * q:32 * q + 32,
                                  512 * h2:512 * (h2 + 1)],
                                lt[:, 32 * (c // 2):32 * (c // 2) + 32],
                                wt[c][:, cs:cs + 512],
                                start=False, stop=(c == 7),
                                skip_group_check=True,
                                tile_position=(0, 32 * q))

                # one projection granule per step fills the PE idle slot
                # left by the elementwise chain below
                g = n_pro + t
                if g < n_gran:
                    proj_granule(g)

                Hs = hsp.tile([128, 256], F16, tag="Hs")
                LE = lhsp.tile([128, 128], F16, tag="LE")
                LO = lhsp.tile([128, 128], F16, tag="LO")
                for hf in range(2):
                    pcol = 512 * hf
                    Sf = selem.tile([128, 384], F16, tag=f"S{hf}")
                    nc.scalar.activation(Sf[:], P[:, pcol:pcol + 384], SIG)
                    Sg = selem.tile([128, 128], F16, tag=f"Sg{hf}")
                    nc.scalar.activation(
                        Sg[:], P[:, pcol + 384:pcol + 512], TANH)
                    U = selem.tile([128, 128], F16, tag=f"U{hf}")
                    nc.vector.scalar_tensor_tensor(
                        U[:], Sf[:, 128:256], 1.0, Sg[:], byp, mult)
                    ch = cst[:, 128 * hf:128 * (hf + 1)]
                    nc.vector.scalar_tensor_tensor(
                        ch, Sf[:, 0:128], 1.0, ch, byp, mult)
                    nc.vector.scalar_tensor_tensor(
                        ch, ch, 1.0, U[:], byp, add)
                    Tc = selem.tile([128, 128], F16, tag=f"Tc{hf}")
                    nc.scalar.activation(Tc[:], ch, TANH)
                    hh = Hs[:, 128 * hf:128 * (hf + 1)]
                    nc.vector.scalar_tensor_tensor(
                        hh, Sf[:, 256:384], 1.0, Tc[:], byp, mult)
                    # transpose this half -> lhsT for next step
                    TT = tps.tile([128, 128], F16, tag=f"T{hf}")
                    nc.tensor.transpose(TT[:], hh, ident[:])
                    dst = LE if hf == 0 else LO
                    nc.vector.tensor_copy(dst[:], TT[:])

                # write hidden state of this step to DRAM (one DMA per
                # column-group: partition slices must be contiguous)
                for q in range(4):
                    nc.sync.dma_start(hs[t, q], Hs[32 * q:32 * q + 8, :])

    nc.compile()
    return nc


_CACHE = {}


def _get_nc(steps=S, use_bias=False):
    key = (steps, use_bias)
    if key not in _CACHE:
        _CACHE[key] = build_nc(steps, use_bias)
    return _CACHE[key]


# Device gate-column permutation: col = 1024q + 512*half + 128*gate + j,
# H_global = 256q + 128*half + j
_COL = np.arange(GC)
_COL_Q = _COL // 1024
_COL_HALF = (_COL % 1024) // 512
_COL_GATE = (_COL % 512) // 128
_COL_J = _COL % 128
_COL_H = 256 * _COL_Q + 128 * _COL_HALF + _COL_J


def _prep_core(core, x, weight_ih, weight_hh, bias, h0_f, h0_b, c0_f, c0_b,
               steps=S, use_bias=False):
    d = core // 4
    qt = core % 4
    fwd = (d == 0)

    xs = np.asarray(x[8 * qt:8 * qt + 8, :steps], np.float32)       # (8,steps,D)
    if not fwd:
        xs = xs[:, ::-1]
    xT = np.ascontiguousarray(xs.transpose(2, 1, 0).reshape(D, B * steps))

    wih = np.asarray(weight_ih[4 * d:4 * d + 4], np.float32)        # (4,D,H)
    whh = np.asarray(weight_hh[4 * d:4 * d + 4], np.float32)
    bi = np.asarray(bias[4 * d:4 * d + 4, 0], np.float32)           # (4,H)

    wproj = np.ascontiguousarray(wih[_COL_GATE, :, _COL_H].T)       # (D,GC)
    whh_dev = np.ascontiguousarray(whh[_COL_GATE, :, _COL_H].T)     # (H,GC)
    bias_dev = np.ascontiguousarray(bi[_COL_GATE, _COL_H])[None, :]

    h0 = np.asarray(h0_f if fwd else h0_b, np.float32).reshape(H)
    c0 = np.asarray(c0_f if fwd else c0_b, np.float32).reshape(H)

    h0T = np.zeros((128, 256), np.float32)
    p = np.arange(128)
    for q in range(4):
        for b in range(8):
            h0T[:, 32 * q + b] = h0[256 * q + p]
            h0T[:, 128 + 32 * q + b] = h0[256 * q + 128 + p]

    c0_dev = np.zeros((128, 256), np.float32)
    j = np.arange(128)
    for q in range(4):
        for b in range(8):
            c0_dev[32 * q + b, 0:128] = c0[256 * q + j]
            c0_dev[32 * q + b, 128:256] = c0[256 * q + 128 + j]

    i832 = np.zeros((8, 32), np.float32)
    i832[np.arange(8), np.arange(8)] = 1.0

    out = {
        "xT": xT.astype(np.float16),
        "wproj": wproj.astype(np.float16),
        "whh": whh_dev.astype(np.float16),
        "h0T": h0T.astype(np.float16),
        "c0": c0_dev,
        "i832": i832.astype(np.float16),
    }
    if use_bias:
        out["bias"] = bias_dev.astype(np.float16)
    return out


def prep_in_maps(x, mask, weight_ih, weight_hh, bias, h0_f, h0_b, c0_f, c0_b,
                 steps=S):
    use_bias = bool(np.any(np.asarray(bias)))
    return [
        _prep_core(c, x, weight_ih, weight_hh, bias, h0_f, h0_b, c0_f, c0_b,
                   steps, use_bias)
        for c in range(NCORES)
    ]


def assemble(results, mask, steps=S):
    """results: list of 8 dicts with 'hs' (steps,4,B,256)."""
    hidden = np.zeros((N, steps, 2 * H), np.float32)
    for core in range(NCORES):
        d = core // 4
        qt = core % 4
        arr = np.asarray(results[core]["hs"]).astype(np.float32)  # (steps,4,8,256)
        part = arr.transpose(2, 0, 1, 3).reshape(B, steps, H)
        if d == 1:
            part = part[:, ::-1]
        hidden[8 * qt:8 * qt + 8, :, H * d:H * (d + 1)] = part
    final = np.concatenate([hidden[:, -1, :H], hidden[:, 0, H:]], axis=1)
    hidden = hidden * np.asarray(mask, np.float32)[:, :steps, None]
    return hidden.astype(np.float32), final.astype(np.float32)


def kernel(x, mask, weight_ih, weight_hh, bias, h0_f, h0_b, c0_f, c0_b):
    use_bias = bool(np.any(np.asarray(bias)))
    nc = _get_nc(S, use_bias)
    in_maps = prep_in_maps(x, mask, weight_ih, weight_hh, bias,
                           h0_f, h0_b, c0_f, c0_b, S)
    res = bass_utils.run_bass_kernel_spmd(nc, in_maps,
                                          core_ids=list(range(NCORES)))
    return assemble(res.results, mask, S)


# revision 14
# speedup vs baseline: 1.5051x; 1.3240x over previous
"""BiLSTM encoder kernel for 8 Trainium2 NeuronCores.

Problem: N=32, S=256, D=1024, H=1024 bidirectional LSTM.

Sharding (no cross-core communication):
  8 cores = 2 directions x 4 batch-quarters (8 batch rows each).
  Each core:
    Phase 1 (proj): x_h[token, col] = x^T-stationary @ W_ih-streaming  (+bias)
    Phase 2 (scan): 256 sequential LSTM steps.
      Matmul orientation: h^T stationary ([128,8] per K-chunk), W_hh streams
      through 4 concurrent PE column-groups (tile_position) -> stream-bound.
      x_h[t] enters the same PSUM accumulation via a K=8 identity matmul.
      Elementwise on junk-padded [128, F] tiles (cost is free-dim driven).
      h' is re-transposed for the next step with 2 PE transposes.

Device gate-column permutation: col = 1024*q + 512*half + 128*gate + j
  with H_global = 256*q + 128*half + j  (q: PE col-group, gate: f,i,o,g).

All 8 cores run an identical program; direction/batch specialization is
done host-side (time-reversed inputs for the backward cores).
"""

import os
import sys

for _p in ("/opt/trn_rl_repo", "/root/.axon_site/_ro/trn_rl_repo"):
    if os.path.isdir(_p) and _p not in sys.path:
        sys.path.insert(0, _p)

import numpy as np

import concourse.bass as bass
import concourse.mybir as mybir
from concourse import bacc
from concourse import bass_utils
from concourse.tile import TileContext
from concourse.masks import make_identity

N, S, D, H = 32, 256, 1024, 1024
B = 8            # batch rows per core
NCORES = 8
TOK = B * S      # tokens per core (t-major: token = t*B + b)
GC = 4 * H       # gate columns per direction (f,i,o,g x H)

F32 = mybir.dt.float32
F16 = mybir.dt.float16


def build_nc(steps=S, use_bias=False):
    """Build and schedule the SPMD program (identical on all 8 cores).

    Single fused phase: the input projection is emitted as small
    "granules" (one token-block x 512-col block) interleaved into the
    scan loop, so they fill PE idle slots while the per-step elementwise
    chain runs, and keep the PE HAM-warm.
    """
    nc = bacc.Bacc(
        "TRN2",
        target_bir_lowering=False,
        debug=False,
        enable_asserts=False,
        num_devices=NCORES,
    )

    ntok = B * steps
    xT = nc.dram_tensor("xT", [D, ntok], F16, kind="ExternalInput").ap()
    wproj = nc.dram_tensor("wproj", [D, GC], F16, kind="ExternalInput").ap()
    if use_bias:
        bias_d = nc.dram_tensor("bias", [1, GC], F16, kind="ExternalInput").ap()
    whh = nc.dram_tensor("whh", [H, GC], F16, kind="ExternalInput").ap()
    h0T = nc.dram_tensor("h0T", [128, 256], F16, kind="ExternalInput").ap()
    c0 = nc.dram_tensor("c0", [128, 256], F32, kind="ExternalInput").ap()
    i832_d = nc.dram_tensor("i832", [8, 32], F16, kind="ExternalInput").ap()
    hs = nc.dram_tensor("hs", [steps, 4, B, 256], F16, kind="ExternalOutput").ap()

    byp = mybir.AluOpType.bypass
    mult = mybir.AluOpType.mult
    add = mybir.AluOpType.add
    SIG = mybir.ActivationFunctionType.Sigmoid
    TANH = mybir.ActivationFunctionType.Tanh

    n_tb = ntok // 128          # 128-token stationary blocks
    n_gran = n_tb * 8           # granules: (tb, hf, cb) with 512 cols each

    with TileContext(nc) as tc:
        with tc.tile_pool(name="dram", bufs=1, space="DRAM") as dpool, \
             tc.tile_pool(name="wts", bufs=1) as wts, \
             tc.tile_pool(name="sst", bufs=1) as sst, \
             tc.tile_pool(name="xhp", bufs=2) as xhp, \
             tc.tile_pool(name="gps", bufs=2, space="PSUM") as gps, \
             tc.tile_pool(name="tps", bufs=1, space="PSUM") as tps, \
             tc.tile_pool(name="pps", bufs=2, space="PSUM") as pps, \
             tc.tile_pool(name="psb", bufs=2) as psb, \
             tc.tile_pool(name="selem", bufs=2) as selem, \
             tc.tile_pool(name="hsp", bufs=3) as hsp, \
             tc.tile_pool(name="lhsp", bufs=2) as lhsp:

            xh_dram = dpool.tile([ntok, GC], F16, tag="xh")

            # ---- static tiles / weight loads ----
            xt_t = []
            for d in range(8):
                t = wts.tile([128, ntok], F16, tag=f"xt{d}")
                nc.sync.dma_start(t[:], xT[128 * d:128 * (d + 1), :])
                xt_t.append(t)
            wp_t = []
            for d in range(8):
                t = wts.tile([128, GC], F16, tag=f"wp{d}")
                nc.sync.dma_start(t[:], wproj[128 * d:128 * (d + 1), :])
                wp_t.append(t)
            wt = []
            for c in range(8):
                t = wts.tile([128, GC], F16, tag=f"whh{c}")
                nc.sync.dma_start(t[:], whh[128 * c:128 * (c + 1), :])
                wt.append(t)
            if use_bias:
                ones1 = sst.tile([1, 128], F16, tag="ones1")
                nc.vector.memset(ones1[:], 1.0)
                bias_sb = sst.tile([1, GC], F16, tag="biassb")
                nc.sync.dma_start(bias_sb[:], bias_d[:])
            i832 = sst.tile([8, 32], F16, tag="i832")
            nc.sync.dma_start(i832[:], i832_d[:])
            ident = sst.tile([128, 128], F16, tag="ident")
            make_identity(nc, ident[:])
            cst = sst.tile([128, 256], F32, tag="cst")
            nc.sync.dma_start(cst[:], c0[:])

            LE = lhsp.tile([128, 128], F16, tag="LE")
            LO = lhsp.tile([128, 128], F16, tag="LO")
            nc.sync.dma_start(LE[:], h0T[:, 0:128])
            nc.sync.dma_start(LO[:], h0T[:, 128:256])

            def proj_granule_mm(g):
                tb = g // 8
                hf = (g % 8) // 4
                cb = g % 4
                col = 2048 * hf + 512 * cb
                pp = pps.tile([128, 512], F32, tag="pp")
                for d in range(8):
                    nc.tensor.matmul(
                        pp[:], xt_t[d][:, 128 * tb:128 * (tb + 1)],
                        wp_t[d][:, col:col + 512],
                        start=(d == 0), stop=(d == 7 and not use_bias))
                if use_bias:
                    nc.tensor.matmul(
                        pp[:], ones1[:, :], bias_sb[:, col:col + 512],
                        start=False, stop=True)
                return pp

            def proj_granule_drain(g, pp):
                tb = g // 8
                hf = (g % 8) // 4
                cb = g % 4
                col = 2048 * hf + 512 * cb
                sb = psb.tile([128, 512], F16, tag="projsb")
                nc.scalar.activation(
                    sb[:], pp[:], mybir.ActivationFunctionType.Copy)
                nc.sync.dma_start(
                    xh_dram[128 * tb:128 * (tb + 1), col:col + 512], sb[:])

            def proj_granule(g):
                proj_granule_drain(g, proj_granule_mm(g))

            # prologue: first two token blocks of x_h
            n_pro = min(16, n_gran)
            for g in range(n_pro):
                proj_granule(g)

            def elem_half(P_half, hf, Hs, dst):
                """Elementwise chain for one H-half + its transpose."""
                Sf = selem.tile([128, 384], F16, tag=f"S{hf}")
                nc.scalar.activation(Sf[:], P_half[:, 0:384], SIG)
                Sg = selem.tile([128, 128], F16, tag=f"Sg{hf}")
                nc.scalar.activation(Sg[:], P_half[:, 384:512], TANH)
                U = selem.tile([128, 128], F16, tag=f"U{hf}")
                nc.vector.scalar_tensor_tensor(
                    U[:], Sf[:, 128:256], 1.0, Sg[:], byp, mult)
                ch = cst[:, 128 * hf:128 * (hf + 1)]
                nc.vector.scalar_tensor_tensor(
                    ch, Sf[:, 0:128], 1.0, ch, byp, mult)
                nc.vector.scalar_tensor_tensor(
                    ch, ch, 1.0, U[:], byp, add)
                Tc = selem.tile([128, 128], F16, tag=f"Tc{hf}")
                nc.scalar.activation(Tc[:], ch, TANH)
                hh = Hs[:, 128 * hf:128 * (hf + 1)]
                nc.vector.scalar_tensor_tensor(
                    hh, Sf[:, 256:384], 1.0, Tc[:], byp, mult)
                # transpose this half -> lhsT for next step
                TT = tps.tile([128, 128], F16, tag=f"T{hf}")
                nc.tensor.transpose(TT[:], hh, ident[:])
                nc.vector.tensor_copy(dst[:], TT[:])

            for t in range(steps):
                xh_t = xhp.tile([8, GC], F16, tag="xht")
                nc.sync.dma_start(xh_t[:], xh_dram[B * t:B * (t + 1), :])

                # Per-half PSUM tiles so each half's elementwise chain can
                # start as soon as that half's matmuls finish (tile-level
                # dependency tracking). MM order: col-group q innermost so
                # the 4 PE column groups stream concurrently; even K-chunks
                # (needing only LE) before odd ones. Every MM writes all 32
                # rows of its group block so accumulation groups are
                # well-formed.
                PH = []
                for h2 in range(2):
                    P = gps.tile([128, 512], F32, tag=f"P{h2}")
                    PH.append(P)
                    for q in range(4):
                        cs = 1024 * q + 512 * h2
                        nc.tensor.matmul(
                            P[32 * q:32 * q + 32, :],
                            i832[:, :], xh_t[:, cs:cs + 512],
                            start=True, stop=False,
                            skip_group_check=True,
                            tile_position=(0, 32 * q))
                    for c in (0, 2, 4, 6, 1, 3, 5, 7):
                        lt = LE if (c % 2 == 0) else LO
                        for q in range(4):
                            cs = 1024 * q + 512 * h2
                            nc.tensor.matmul(
                                P[32 * q:32 * q + 32, :],
                                lt[:, 32 * (c // 2):32 * (c // 2) + 32],
                                wt[c][:, cs:cs + 512],
                                start=False, stop=(c == 7),
                                skip_group_check=True,
                                tile_position=(0, 32 * q))

                Hs = hsp.tile([128, 256], F16, tag="Hs")
                LE = lhsp.tile([128, 128], F16, tag="LE")
                LO = lhsp.tile([128, 128], F16, tag="LO")

                # one granule every other step: its MMs fill the PE wait
                # for the second half's chain; drain happens off-chain
                g = n_pro + t // 2
                do_gran = (t % 2 == 0) and g < n_gran
                elem_half(PH[0], 0, Hs, LE)      # ...TTA on PE
                pp = proj_granule_mm(g) if do_gran else None
                elem_half(PH[1], 1, Hs, LO)      # ...TTB on PE
                if do_gran:
                    proj_granule_drain(g, pp)

                # write hidden state of this step to DRAM (one DMA per
                # column-group: partition slices must be contiguous)
                for q in range(4):
                    nc.sync.dma_start(hs[t, q], Hs[32 * q:32 * q + 8, :])

    nc.compile()
    return nc


_CACHE = {}


def _get_nc(steps=S, use_bias=False):
    key = (steps, use_bias)
    if key not in _CACHE:
        _CACHE[key] = build_nc(steps, use_bias)
    return _CACHE[key]


# Device gate-column permutation: col = 1024q + 512*half + 128*gate + j,
# H_global = 256q + 128*half + j
_COL = np.arange(GC)
_COL_Q = _COL // 1024
_COL_HALF = (_COL % 1024) // 512
_COL_GATE = (_COL % 512) // 128
_COL_J = _COL % 128
_COL_H = 256 * _COL_Q + 128 * _COL_HALF + _COL_J


def _prep_core(core, x, weight_ih, weight_hh, bias, h0_f, h0_b, c0_f, c0_b,
               steps=S, use_bias=False):
    d = core // 4
    qt = core % 4
    fwd = (d == 0)

    xs = np.asarray(x[8 * qt:8 * qt + 8, :steps], np.float32)       # (8,steps,D)
    if not fwd:
        xs = xs[:, ::-1]
    xT = np.ascontiguousarray(xs.transpose(2, 1, 0).reshape(D, B * steps))

    wih = np.asarray(weight_ih[4 * d:4 * d + 4], np.float32)        # (4,D,H)
    whh = np.asarray(weight_hh[4 * d:4 * d + 4], np.float32)
    bi = np.asarray(bias[4 * d:4 * d + 4, 0], np.float32)           # (4,H)

    wproj = np.ascontiguousarray(wih[_COL_GATE, :, _COL_H].T)       # (D,GC)
    whh_dev = np.ascontiguousarray(whh[_COL_GATE, :, _COL_H].T)     # (H,GC)
    bias_dev = np.ascontiguousarray(bi[_COL_GATE, _COL_H])[None, :]

    h0 = np.asarray(h0_f if fwd else h0_b, np.float32).reshape(H)
    c0 = np.asarray(c0_f if fwd else c0_b, np.float32).reshape(H)

    h0T = np.zeros((128, 256), np.float32)
    p = np.arange(128)
    for q in range(4):
        for b in range(8):
            h0T[:, 32 * q + b] = h0[256 * q + p]
            h0T[:, 128 + 32 * q + b] = h0[256 * q + 128 + p]

    c0_dev = np.zeros((128, 256), np.float32)
    j = np.arange(128)
    for q in range(4):
        for b in range(8):
            c0_dev[32 * q + b, 0:128] = c0[256 * q + j]
            c0_dev[32 * q + b, 128:256] = c0[256 * q + 128 + j]

    i832 = np.zeros((8, 32), np.float32)
    i832[np.arange(8), np.arange(8)] = 1.0

    out = {
        "xT": xT.astype(np.float16),
        "wproj": wproj.astype(np.float16),
        "whh": whh_dev.astype(np.float16),
        "h0T": h0T.astype(np.float16),
        "c0": c0_dev,
        "i832": i832.astype(np.float16),
    }
    if use_bias:
        out["bias"] = bias_dev.astype(np.float16)
    return out


def prep_in_maps(x, mask, weight_ih, weight_hh, bias, h0_f, h0_b, c0_f, c0_b,
                 steps=S):
    use_bias = bool(np.any(np.asarray(bias)))
    return [
        _prep_core(c, x, weight_ih, weight_hh, bias, h0_f, h0_b, c0_f, c0_b,
                   steps, use_bias)
        for c in range(NCORES)
    ]


def assemble(results, mask, steps=S):
    """results: list of 8 dicts with 'hs' (steps,4,B,256)."""
    hidden = np.zeros((N, steps, 2 * H), np.float32)
    for core in range(NCORES):
        d = core // 4
        qt = core % 4
        arr = np.asarray(results[core]["hs"]).astype(np.float32)  # (steps,4,8,256)
        part = arr.transpose(2, 0, 1, 3).reshape(B, steps, H)
        if d == 1:
            part = part[:, ::-1]
        hidden[8 * qt:8 * qt + 8, :, H * d:H * (d + 1)] = part
    final = np.concatenate([hidden[:, -1, :H], hidden[:, 0, H:]], axis=1)
    hidden = hidden * np.asarray(mask, np.float32)[:, :steps, None]
    return hidden.astype(np.float32), final.astype(np.float32)


def kernel(x, mask, weight_ih, weight_hh, bias, h0_f, h0_b, c0_f, c0_b):
    use_bias = bool(np.any(np.asarray(bias)))
    nc = _get_nc(S, use_bias)
    in_maps = prep_in_maps(x, mask, weight_ih, weight_hh, bias,
                           h0_f, h0_b, c0_f, c0_b, S)
    res = bass_utils.run_bass_kernel_spmd(nc, in_maps,
                                          core_ids=list(range(NCORES)))
    return assemble(res.results, mask, S)
